# revision 25
# baseline (speedup 1.0000x reference)
"""PrRoIPool (Precise RoI Pooling) on 8 Trainium2 NeuronCores.

Strategy
--------
Host: ROIs are routed to cores grouped by their batch index, so every core
receives ROIs of a single image; that core's input carries only that image's
feature slab (the feature map is replicated per the sharding hint, but each
core only needs its own batch's slab). Up to CAP=40 ROI slots per core
(ceil(n0/40) + ceil(n1/40) <= 8 for any split of 256 ROIs over 2 images).

Device (per core, identical SPMD program):
  1. Load feature slab f[c,h,w] into SBUF as [h, c, w] (h on partitions).
  2. Compute the exact hat-kernel integral weights on device from the raw
     ROI boxes: WyT[h,(g,p)] and WxT[w,(g,q)] via the hat-CDF
     phi(t) = 0.5*(1 + sign(t)*(1 - relu(1-|t|)^2)).
  3. Stage 1 (PE): for each channel c, matmul with stationary f[:,c,:]
     ([h,w]) and moving WyT -> t[w, (g,p)] per c.  This puts w on the
     partition axis so stage 2 can contract over it.
  4. Stage 2 (PE): per ROI g, matmul stationary WxT[:,7g:7g+7] ([w,q]) with
     moving t[w, (c,p)] -> out[q, (c,p)], 4 ROIs col-tiled per PSUM bank.
  5. Scale by 1/area during the PSUM->SBUF copy (per-partition scalar),
     DMA out as [chunk, j, q, c, p]; host permutes axes and un-sorts ROIs.
"""

import sys

if "/opt/trn_rl_repo" not in sys.path:
    sys.path.insert(0, "/opt/trn_rl_repo")

import math
import os

import numpy as np

import concourse.bass as bass
import concourse.mybir as mybir
import concourse.tile as tile
from concourse import bacc
from concourse.bass_utils import run_bass_kernel_spmd

B, C, H, W = 2, 64, 100, 100
OUT = 7
E = OUT + 1
SCALE = 0.25
N_ROIS = 256
CAP = 40  # per-core ROI slots
NCH = CAP // 4  # output chunks of 4 col-tiled ROIs
N_CORES = 8
GP = CAP * OUT  # 280 = stage-1 moving columns
CP = C * OUT  # 448 = stage-2 moving columns

F32 = mybir.dt.float32
I32 = mybir.dt.int32

_CACHE = {}


def _build_module():
    use_f32r = os.environ.get("KERNEL_MM_DTYPE", "f32r") == "f32r"
    MMD = mybir.dt.float32r if use_f32r else F32
    BF16 = mybir.dt.bfloat16

    def mm(ap):
        return ap

    AF = mybir.ActivationFunctionType
    nc = bacc.Bacc("TRN2")
    feat = nc.dram_tensor("feat", [C, H, W], F32, kind="ExternalInput")
    roisd = nc.dram_tensor("rois", [CAP * 4], F32, kind="ExternalInput")
    outd = nc.dram_tensor("out", [NCH, 4, OUT, C, OUT], F32, kind="ExternalOutput")

    with tile.TileContext(nc) as tc:
        with (
            tc.tile_pool(name="const", bufs=1) as cons,
            tc.tile_pool(name="feats", bufs=1) as featp,
            tc.tile_pool(name="tbuf", bufs=1) as tbufp,
            tc.tile_pool(name="outs", bufs=1) as outp,
        ):
            # ---------- per-ROI parameter broadcast ----------
            rt = roisd.ap().tensor
            rb = cons.tile([128, CAP, 4], F32)
            nc.gpsimd.dma_start(
                out=rb[:],
                in_=bass.AP(tensor=rt, offset=0, ap=[[0, 128], [1, CAP * 4]]).rearrange(
                    "p (g k) -> p g k", k=4
                ),
            )

            # ---------- feature load (h on partitions) ----------
            fsb = featp.tile([128, C, W], MMD)
            fr = feat.ap().bitcast(MMD).rearrange("c h w -> h c w")
            CCH = 16
            for c0 in range(0, C, CCH):
                nc.sync.dma_start(
                    out=fsb[0:H, c0 : c0 + CCH, :], in_=fr[:, c0 : c0 + CCH, :]
                )

            # ---------- bin sizes, origins, edge grids u = scale*lo + e*bw ---
            bins = cons.tile([128, 2, CAP], F32)  # bw, bh
            orig = cons.tile([128, 2, CAP], F32)  # scale*x1, scale*y1
            for ax in range(2):
                d = cons.tile([128, CAP], F32, tag="dtmp")
                nc.vector.tensor_sub(d[:], rb[:, :, 2 + ax], rb[:, :, ax])
                nc.scalar.activation(bins[:, ax], d[:], AF.Relu, scale=SCALE / OUT)
                nc.scalar.activation(orig[:, ax], rb[:, :, ax], AF.Copy, scale=SCALE)

            def erep(t2):
                # [128, 2, CAP] viewed as [128, 2, CAP, E] via step-0 inner dim
                a = t2[:].ap
                return bass.AP(
                    tensor=t2.tensor,
                    offset=t2.offset,
                    ap=[list(a[0]), list(a[1]), list(a[2]), [0, E]],
                )

            eri = cons.tile([128, 2, CAP, E], I32)
            nc.gpsimd.iota(eri, pattern=[[0, 2], [0, CAP], [1, E]], channel_multiplier=0)
            erf = cons.tile([128, 2, CAP, E], F32)
            nc.vector.tensor_copy(erf[:], eri[:])
            U = cons.tile([128, 2, CAP, E], F32)
            nc.vector.tensor_mul(U[:], erf[:], erep(bins))
            nc.vector.tensor_add(U[:], U[:], erep(orig))

            # ---------- hat-CDF phi, reduced form ----------
            # phi(t) = 0.5 - 0.5*sign(t)*q, q = c1^2 - 2*c1, c1 = clip(|t|,0,1)
            # The bin weight is phi[e+1]-phi[e] = 0.5*(R[e]-R[e+1]), R = sign*q.
            # The two 0.5 factors (x and y axes) are folded into rT (0.25).
            hcol_i = cons.tile([128, 1], I32)
            nc.gpsimd.iota(hcol_i, pattern=[[0, 1]], channel_multiplier=1)
            hcol = cons.tile([128, 1], F32)
            nc.vector.tensor_copy(hcol[:], hcol_i[:])

            NE = 2 * CAP * E
            Uf = U[:].rearrange("p a g e -> p (a g e)")
            T = cons.tile([128, NE], F32)
            nc.vector.tensor_scalar(
                T[:], Uf, hcol[:], None, op0=mybir.AluOpType.subtract
            )
            Aab = cons.tile([128, NE], F32, tag="phi_a")
            nc.scalar.activation(Aab[:], T[:], AF.Abs)
            C1 = cons.tile([128, NE], F32, tag="phi_c1")
            nc.vector.tensor_scalar_min(C1[:], Aab[:], 1.0)
            G = cons.tile([128, NE], F32, tag="phi_g")
            nc.scalar.activation(G[:], T[:], AF.Sign)
            Q = cons.tile([128, NE], F32, tag="phi_q")
            nc.vector.scalar_tensor_tensor(
                Q[:], C1[:], 2.0, C1[:],
                op0=mybir.AluOpType.subtract, op1=mybir.AluOpType.mult,
            )
            R = cons.tile([128, 2, CAP, E], F32)
            nc.vector.tensor_mul(
                R[:].rearrange("p a g e -> p (a g e)"), G[:], Q[:]
            )

            # ---------- axis weights: W[e] = R[e] - R[e+1]  (x2 the true
            # weight; compensated in rT).  WxT padded to 32 cols per ROI so
            # stage-2 matmuls write full 32-partition strips.
            WxT = cons.tile([128, CAP, 32], BF16)
            # zero-fill via a DVE op (memset cannot produce rounded f32r)
            hb = hcol[:].ap
            nc.vector.tensor_scalar_mul(
                WxT[:].rearrange("p g k -> p (g k)"),
                bass.AP(tensor=hcol.tensor, offset=hcol.offset,
                        ap=[list(hb[0]), [0, CAP * 32]]),
                0.0,
            )
            nc.vector.tensor_sub(
                WxT[:, :, 0:OUT], R[:, 0, :, 0:OUT], R[:, 0, :, 1:E]
            )
            WyT = cons.tile([128, CAP, OUT], MMD)
            nc.vector.tensor_sub(WyT[:], R[:, 1, :, 0:OUT], R[:, 1, :, 1:E])

            # ---------- 1/area, arranged per (j, chunk) ----------
            area = cons.tile([128, CAP], F32, tag="ar3")
            nc.vector.tensor_mul(area[:], bins[:, 0], bins[:, 1])
            nc.vector.tensor_scalar_max(area[:], area[:], 1e-12)
            recip = cons.tile([128, CAP], F32, tag="ar4")
            nc.vector.reciprocal(recip[:], area[:])
            # rT[(j,q)-partition, chunk] = 0.25 / area[4*chunk+j]; the 0.25
            # compensates the dropped 0.5 factors of the two axis weights.
            rT = cons.tile([128, NCH], F32)
            for j in range(4):
                nc.vector.tensor_scalar_mul(
                    rT[32 * j : 32 * j + 32, :],
                    recip[32 * j : 32 * j + 32, j : CAP : 4],
                    0.25,
                )

            tsb = tbufp.tile([128, CAP, C, OUT], BF16)
            out2 = outp.tile([128, NCH, CP], F32)

            phase = os.environ.get("KERNEL_PHASE", "full")
            if phase == "params":
                nc.vector.memset(tsb[:], 0.0)
            with tc.tile_pool(name="psA", bufs=2, space="PSUM") as psA:
                # ---- PE warm-up: dummy matmuls during the DMA/param phase
                # keep the PE busy so the HAM clock gate is released before
                # stage 1 starts (and the cost model's p-state ramps up).
                wps = psA.tile([128, 4, 512], F32, tag="ps1")
                for wu in range(8):
                    nc.tensor.matmul(
                        wps[0:25, wu % 4, 0:112],
                        rb[0:100, 0:25, 0],
                        rb[0:100, :, :].rearrange("p g k -> p (g k)")[:, 0:112],
                        start=True,
                        stop=True,
                    )

                # ---- stage 1: t[w, (g,p)] per channel ----------
                WyTf = WyT[0:H].rearrange("p g q -> p (g q)")
                for k in range(C // 4 if phase != "params" else 0):
                    ps1 = psA.tile([128, 4, 512], F32, tag="ps1")
                    for j4 in range(4):
                        c = 4 * k + j4
                        nc.tensor.matmul(
                            ps1[0:W, j4, 0:GP],
                            mm(fsb[0:H, c, :]),
                            mm(WyTf),
                            start=True,
                            stop=True,
                        )
                    dst = bass.AP(
                        tensor=tsb.tensor,
                        offset=tsb.offset + 4 * k * OUT,
                        ap=[list(tsb[:].ap[0][:1]) + [100]]
                        + [[OUT, 4], [C * OUT, CAP], [1, OUT]],
                    )
                    src_ap = ps1[0:W, :, 0:GP].rearrange(
                        "w b (g p) -> w b g p", p=OUT
                    )
                    if k % 2 == 0:
                        nc.vector.tensor_copy(dst, src_ap)
                    else:
                        nc.scalar.copy(dst, src_ap)

            # ---------- stage 2: out[q, (c,p)] per ROI, 4 ROIs per bank ----
            if phase != "full":
                nc.vector.memset(out2[:], 0.0)
            with tc.tile_pool(name="psB", bufs=2, space="PSUM") as psB:
                for ch in range(NCH if phase == "full" else 0):
                    ps2 = psB.tile([128, CP], F32, tag="ps2")
                    for j in range(4):
                        g = 4 * ch + j
                        nc.tensor.matmul(
                            ps2[32 * j : 32 * j + 32, :],
                            mm(WxT[0:W, g, :]),
                            mm(tsb[0:W, g, :, :].rearrange("w c p -> w (c p)")),
                            start=True,
                            stop=True,
                            tile_position=(0, 32 * j),
                        )
                    if ch % 2 == 0:
                        nc.vector.tensor_scalar(
                            out2[:, ch, :],
                            ps2[:],
                            rT[:, ch : ch + 1],
                            None,
                            op0=mybir.AluOpType.mult,
                        )
                    else:
                        nc.scalar.activation(
                            out2[:, ch, :],
                            ps2[:],
                            AF.Copy,
                            scale=rT[:, ch : ch + 1],
                        )

            # ---------- store ----------
            od = outd.ap().rearrange("n j q c p -> j q n (c p)")
            for j in range(4):
                nc.sync.dma_start(
                    out=od[j],
                    in_=out2[32 * j : 32 * j + OUT, :, :],
                )

    nc.finalize()
    return nc


def _get_module():
    if "nc" not in _CACHE:
        _CACHE["nc"] = _build_module()
    return _CACHE["nc"]


def _shard_rois(bidx):
    """Assign ROI indices (grouped by batch) to cores; returns per-core
    (batch, index-list)."""
    groups = [np.nonzero(bidx == b)[0] for b in range(B)]
    counts = [len(g) for g in groups]
    need = [math.ceil(c / CAP) for c in counts]
    assert sum(need) <= N_CORES, (counts, "cannot shard")
    k = list(need)
    extra = N_CORES - sum(k)
    while extra > 0:
        loads = [counts[b] / k[b] if k[b] > 0 else -1.0 for b in range(B)]
        b = int(np.argmax(loads))
        if loads[b] <= 0:
            break
        k[b] += 1
        extra -= 1
    assign = []
    for b in range(B):
        if k[b] == 0:
            continue
        parts = np.array_split(groups[b], k[b])
        for p in parts:
            assign.append((b, p))
    while len(assign) < N_CORES:
        assign.append((0, np.zeros((0,), np.int64)))
    return assign[:N_CORES]


def kernel(features, rois):
    features = np.ascontiguousarray(np.asarray(features, dtype=np.float32))
    rois = np.asarray(rois, dtype=np.float32)
    assert features.shape == (B, C, H, W) and rois.shape == (N_ROIS, 5)

    bidx = rois[:, 0].astype(np.int32)
    assign = _shard_rois(bidx)

    nc = _get_module()
    in_maps = []
    for b, idxs in assign:
        rc = np.zeros((CAP, 4), np.float32)
        rc[: len(idxs)] = rois[idxs, 1:5]
        in_maps.append(
            {"feat": features[b], "rois": np.ascontiguousarray(rc.reshape(-1))}
        )

    res = run_bass_kernel_spmd(nc, in_maps, list(range(N_CORES)))

    out = np.zeros((N_ROIS, C, OUT, OUT), np.float32)
    for core, (b, idxs) in enumerate(assign):
        if len(idxs) == 0:
            continue
        o = np.asarray(res.results[core]["out"])  # [NCH, 4, q, c, p]
        og = np.transpose(o, (0, 1, 3, 4, 2)).reshape(CAP, C, OUT, OUT)
        out[idxs] = og[: len(idxs)]
    return out


# revision 38
# speedup vs baseline: 1.2635x; 1.2635x over previous
"""PrRoIPool (Precise RoI Pooling) on 8 Trainium2 NeuronCores.

Strategy
--------
Host: ROIs are routed to cores grouped by their batch index, so every core
receives ROIs of a single image; that core's input carries only that image's
feature slab (the hint's "replicate the feature map" specialised to the one
image the core needs).  The per-core ROI capacity CAP adapts to the batch
split (32 for a balanced 128/128 split); the module is compiled per CAP and
cached.

Device (per core, identical SPMD program):
  1. Load feature slab f[c,h,w] into SBUF as [h, c, w] (h on partitions),
     cast to bf16 during the DMA (SWDGE).
  2. Compute the exact hat-kernel integral weights on device from the raw
     ROI boxes via the hat-CDF phi(t) = 0.5 - 0.5*sign(t)*(c1^2-2c1),
     c1 = clip(|t|,0,1); bin weight = phi diff (x2, folded into 1/area).
  3. Stage 1 (PE): per channel c, matmul stationary f[:,c,:] ([h,w]) with
     moving WyT[h,(g,p)] -> t[w, (g,c-major...,p)].  Puts w on partitions.
  4. Stage 2 (PE): per ROI g, matmul stationary WxT[:,g,:] ([w,q]) with
     moving t[w,g,(c,p)] -> out[q,(c,p)], 4 ROIs col-tiled per PSUM bank.
  5. Scale by 0.25/area during the PSUM->SBUF copy (per-partition scalar),
     DMA out as [chunk, j, q, c, p]; host permutes axes and un-sorts ROIs.
"""

import sys

if "/opt/trn_rl_repo" not in sys.path:
    sys.path.insert(0, "/opt/trn_rl_repo")

import math
import os

import numpy as np

import concourse.bass as bass
import concourse.mybir as mybir
import concourse.tile as tile
from concourse import bacc
from concourse.bass_utils import run_bass_kernel_spmd

B, C, H, W = 2, 64, 100, 100
OUT = 7
E = OUT + 1
SCALE = 0.25
N_ROIS = 256
N_CORES = 8
CP = C * OUT  # 448 = stage-2 moving columns
WPAD = 128  # stationary w padded to 128 cols (enables bf16 FWL)

F32 = mybir.dt.float32
I32 = mybir.dt.int32
BF16 = mybir.dt.bfloat16

_CACHE = {}


def _build_module(cap):
    GP = cap * OUT
    NCH = cap // 4
    AF = mybir.ActivationFunctionType
    nc = bacc.Bacc("TRN2")
    feat = nc.dram_tensor("feat", [C, H, W], F32, kind="ExternalInput")
    roisd = nc.dram_tensor("rois", [cap * 4], F32, kind="ExternalInput")
    outd = nc.dram_tensor("out", [NCH, 4, OUT, C, OUT], F32, kind="ExternalOutput")

    with tile.TileContext(nc) as tc:
        with (
            tc.tile_pool(name="const", bufs=1) as cons,
            tc.tile_pool(name="feats", bufs=1) as featp,
            tc.tile_pool(name="tbuf", bufs=1) as tbufp,
            tc.tile_pool(name="outs", bufs=1) as outp,
        ):
            # ---------- per-ROI parameter broadcast ----------
            rt = roisd.ap().tensor
            rb = cons.tile([128, cap, 4], F32)
            nc.sync.dma_start(
                out=rb[:],
                in_=bass.AP(
                    tensor=rt, offset=0, ap=[[0, 128], [1, cap * 4]]
                ).rearrange("p (g k) -> p g k", k=4),
            )

            # ---------- feature load (h on partitions, f32r view) ----------
            FR = mybir.dt.float32r
            fsb = featp.tile([128, C, W], FR)
            fr = feat.ap().bitcast(FR).rearrange("c h w -> h c w")
            CCH = int(os.environ.get("KERNEL_DMA_CHUNK", "8"))
            for i, c0 in enumerate(range(0, C, CCH)):
                eng = nc.scalar if i % 2 == 1 else nc.sync
                eng.dma_start(
                    out=fsb[0:H, c0 : c0 + CCH, :], in_=fr[:, c0 : c0 + CCH, :]
                )

            # ---------- axis weights via hat-CDF ----------
            # phi(t) = 0.5 + 0.5*(A^2 - B^2), A = clip(1+t,0,1),
            # B = clip(1-t,0,1)  (sign-free identity).  Bin weight
            # = phi[e+1]-phi[e] = 0.5*(S[e+1]-S[e]), S = A^2-B^2; the two
            # 0.5 factors (x and y axes) fold into rT (0.25).
            # y axis first (it gates stage 1); x axis computed after.
            hcol_i = cons.tile([128, 1], I32)
            nc.gpsimd.iota(hcol_i, pattern=[[0, 1]], channel_multiplier=1)
            hm1 = cons.tile([128, 1], F32)  # h - 1
            nc.gpsimd.tensor_scalar_sub(hm1[:], hcol_i[:], 1.0)
            hp1 = cons.tile([128, 1], F32)  # h + 1
            nc.gpsimd.tensor_scalar_add(hp1[:], hcol_i[:], 1.0)
            eri = cons.tile([128, cap, E], I32)
            nc.gpsimd.iota(eri, pattern=[[0, cap], [1, E]], channel_multiplier=0)
            erf = cons.tile([128, cap, E], F32)
            nc.gpsimd.tensor_copy(erf[:], eri[:])

            NE1 = cap * E
            GPP = max(256, GP)  # stage-1 moving cols padded for f32r rate
            WyT = cons.tile([128, GPP], mybir.dt.float32r)
            WxT = cons.tile([128, cap, 32], BF16)
            bins_x = cons.tile([128, cap], F32, tag="binsx")
            bins_y = cons.tile([128, cap], F32, tag="binsy")
            binst = [bins_x, bins_y]

            def axis_weights_dve(ax, wout):
                # y-axis chain, all on DVE (gates stage 1)
                lo, hi = (rb[:, :, ax], rb[:, :, 2 + ax])
                d = cons.tile([128, cap], F32, tag=f"d{ax}")
                nc.vector.tensor_sub(d[:], hi, lo)
                nc.vector.tensor_scalar(
                    binst[ax][:], d[:], SCALE / OUT, 0.0,
                    op0=mybir.AluOpType.mult, op1=mybir.AluOpType.max,
                )
                los = cons.tile([128, cap], F32, tag=f"los{ax}")
                nc.vector.tensor_scalar_mul(los[:], lo, SCALE)
                U = cons.tile([128, cap, E], F32, tag=f"u{ax}")
                nc.vector.tensor_mul(U[:], erf[:], erep2c(binst[ax]))
                nc.vector.tensor_add(U[:], U[:], erep2c(los))
                Uf = U[:].rearrange("p g e -> p (g e)")
                A = cons.tile([128, NE1], F32, tag=f"a{ax}")
                nc.vector.tensor_scalar(
                    A[:], Uf, hm1[:], 1.0,
                    op0=mybir.AluOpType.subtract, op1=mybir.AluOpType.min,
                )
                nc.vector.tensor_scalar_max(A[:], A[:], 0.0)
                Bc = cons.tile([128, NE1], F32, tag=f"bc{ax}")
                nc.vector.tensor_scalar(
                    Bc[:], Uf, hp1[:], -1.0,
                    op0=mybir.AluOpType.subtract, op1=mybir.AluOpType.mult,
                )
                nc.vector.tensor_scalar(
                    Bc[:], Bc[:], 0.0, 1.0,
                    op0=mybir.AluOpType.max, op1=mybir.AluOpType.min,
                )
                SB = cons.tile([128, NE1], F32, tag=f"sb{ax}")
                nc.vector.tensor_mul(SB[:], Bc[:], Bc[:])
                S = cons.tile([128, cap, E], F32, tag=f"s{ax}")
                Sf = S[:].rearrange("p g e -> p (g e)")
                nc.vector.scalar_tensor_tensor(
                    Sf, A[:], 1.0, A[:],
                    op0=mybir.AluOpType.mult, op1=mybir.AluOpType.mult,
                )
                nc.vector.tensor_sub(Sf, Sf, SB[:])
                nc.vector.tensor_sub(wout, S[:, :, 1:E], S[:, :, 0:OUT])

            def axis_weights_gp(ax, wout):
                # x-axis chain, all on GPSIMD (hidden under stage 1)
                lo, hi = (rb[:, :, ax], rb[:, :, 2 + ax])
                d = cons.tile([128, cap], F32, tag=f"d{ax}")
                # d = hi - lo via  (lo * -1) + hi  : gpsimd lacks tensor_sub
                dneg = cons.tile([128, cap], F32, tag=f"dn{ax}")
                nc.gpsimd.tensor_scalar_mul(dneg[:], lo, -1.0)
                nc.gpsimd.tensor_add(d[:], dneg[:], hi)
                nc.gpsimd.tensor_scalar(
                    binst[ax][:], d[:], SCALE / OUT, 0.0,
                    op0=mybir.AluOpType.mult, op1=mybir.AluOpType.max,
                )
                los = cons.tile([128, cap], F32, tag=f"los{ax}")
                nc.gpsimd.tensor_scalar_mul(los[:], lo, SCALE)
                U = cons.tile([128, cap, E], F32, tag=f"u{ax}")
                nc.gpsimd.tensor_mul(U[:], erf[:], erep2c(binst[ax]))
                nc.gpsimd.tensor_add(U[:], U[:], erep2c(los))
                Uf = U[:].rearrange("p g e -> p (g e)")
                A = cons.tile([128, NE1], F32, tag=f"a{ax}")
                nc.gpsimd.tensor_scalar(
                    A[:], Uf, hm1[:], 1.0,
                    op0=mybir.AluOpType.subtract, op1=mybir.AluOpType.min,
                )
                nc.gpsimd.tensor_scalar_max(A[:], A[:], 0.0)
                Bc = cons.tile([128, NE1], F32, tag=f"bc{ax}")
                nc.gpsimd.tensor_scalar(
                    Bc[:], Uf, hp1[:], -1.0,
                    op0=mybir.AluOpType.subtract, op1=mybir.AluOpType.mult,
                )
                nc.gpsimd.tensor_scalar(
                    Bc[:], Bc[:], 0.0, 1.0,
                    op0=mybir.AluOpType.max, op1=mybir.AluOpType.min,
                )
                SBn = cons.tile([128, NE1], F32, tag=f"sb{ax}")
                nc.gpsimd.tensor_mul(SBn[:], Bc[:], Bc[:])
                nc.gpsimd.tensor_scalar_mul(SBn[:], SBn[:], -1.0)
                SA = cons.tile([128, NE1], F32, tag=f"sa{ax}")
                nc.gpsimd.tensor_mul(SA[:], A[:], A[:])
                S = cons.tile([128, cap, E], F32, tag=f"s{ax}")
                nc.gpsimd.tensor_add(
                    S[:].rearrange("p g e -> p (g e)"), SA[:], SBn[:]
                )
                Sn = cons.tile([128, cap, E], F32, tag=f"sn{ax}")
                nc.gpsimd.tensor_scalar_mul(
                    Sn[:].rearrange("p g e -> p (g e)"),
                    S[:].rearrange("p g e -> p (g e)"),
                    -1.0,
                )
                nc.gpsimd.tensor_add(wout, S[:, :, 1:E], Sn[:, :, 0:OUT])

            def erep2(t2, ax):
                a = t2[:].ap
                return bass.AP(
                    tensor=t2.tensor,
                    offset=t2.offset + ax * cap,
                    ap=[list(a[0]), [1, cap], [0, E]],
                )

            def erep2c(t1):
                a = t1[:].ap
                return bass.AP(
                    tensor=t1.tensor,
                    offset=t1.offset,
                    ap=[list(a[0]), [1, cap], [0, E]],
                )

            # zero-fill the padded weight tiles (engine ops so bf16/f32r round)
            hb = hcol_i[:].ap
            nc.vector.tensor_scalar_mul(
                WyT[:],
                bass.AP(tensor=hcol_i.tensor, offset=hcol_i.offset,
                        ap=[list(hb[0]), [0, GPP]]),
                0.0,
            )
            nc.gpsimd.tensor_scalar_mul(
                WxT[:].rearrange("p g k -> p (g k)"),
                bass.AP(tensor=hcol_i.tensor, offset=hcol_i.offset,
                        ap=[list(hb[0]), [0, cap * 32]]),
                0.0,
            )
            wyv = WyT[:].ap
            axis_weights_dve(
                1,
                bass.AP(tensor=WyT.tensor, offset=WyT.offset,
                        ap=[list(wyv[0]), [OUT, cap], [1, OUT]]),
            )
            axis_weights_gp(0, WxT[:, :, 0:OUT])

            # ---------- 0.25/area, arranged per (j, chunk) ----------
            area = cons.tile([128, cap], F32, tag="ar3")
            nc.vector.tensor_mul(area[:], binst[0][:], binst[1][:])
            nc.vector.tensor_scalar_max(area[:], area[:], 1e-12)
            recip = cons.tile([128, cap], F32, tag="ar4")
            nc.vector.reciprocal(recip[:], area[:])
            rT = cons.tile([128, NCH], F32)
            for j in range(4):
                nc.vector.tensor_scalar_mul(
                    rT[32 * j : 32 * j + 32, :],
                    recip[32 * j : 32 * j + 32, j : cap : 4],
                    0.25,
                )

            tsb = tbufp.tile([128, cap, C, OUT], BF16)
            out2 = outp.tile([128, NCH, CP], F32)

            phase = os.environ.get("KERNEL_PHASE", "full")
            with tc.tile_pool(name="psA", bufs=2, space="PSUM") as psA:
                # ---- PE warm-up during the DMA/param phase (HAM release)
                wps = psA.tile([128, 4, 512], F32, tag="ps1")
                erff = erf[:].rearrange("p g e -> p (g e)")
                for wu in range(10):
                    nc.tensor.matmul(
                        wps[0:32, wu % 4, 0:128],
                        erff[0:100, 0:32],
                        erff[0:100, 128 : 128 + 128],
                        start=True,
                        stop=True,
                    )

                # ---- stage 1: t[w, (g,p)] per channel ----------
                WyTf = WyT[0:H, :]
                for k in range(C // 4 if phase != "params" else 0):
                    ps1 = psA.tile([128, 4, 512], F32, tag="ps1")
                    for j4 in range(4):
                        c = 4 * k + j4
                        nc.tensor.matmul(
                            ps1[0:W, j4, 0:GPP],
                            fsb[0:H, c, :],
                            WyTf,
                            start=True,
                            stop=True,
                        )
                    dst = bass.AP(
                        tensor=tsb.tensor,
                        offset=tsb.offset + 4 * k * OUT,
                        ap=[list(tsb[:].ap[0][:1]) + [100]]
                        + [[OUT, 4], [C * OUT, cap], [1, OUT]],
                    )
                    src_ap = ps1[0:W, :, 0:GP].rearrange("w b (g p) -> w b g p", p=OUT)
                    if k % 2 == 0:
                        nc.vector.tensor_copy(dst, src_ap)
                    else:
                        nc.scalar.copy(dst, src_ap)

            # ---------- stage 2: out[q, (c,p)] per ROI, 4 ROIs per bank ----
            if phase != "full":
                nc.vector.memset(out2[:], 0.0)
            with tc.tile_pool(name="psB", bufs=4, space="PSUM") as psB:
                for ch in range(NCH if phase == "full" else 0):
                    ps2 = psB.tile([128, CP], F32, tag="ps2")
                    for j in range(4):
                        g = 4 * ch + j
                        nc.tensor.matmul(
                            ps2[32 * j : 32 * j + 32, :],
                            WxT[0:W, g, :],
                            tsb[0:W, g, :, :].rearrange("w c p -> w (c p)"),
                            start=True,
                            stop=True,
                            tile_position=(0, 32 * j),
                        )
                    if ch % 2 == 0:
                        nc.vector.tensor_scalar(
                            out2[:, ch, :],
                            ps2[:],
                            rT[:, ch : ch + 1],
                            None,
                            op0=mybir.AluOpType.mult,
                        )
                    else:
                        nc.scalar.activation(
                            out2[:, ch, :], ps2[:], AF.Copy, scale=rT[:, ch : ch + 1]
                        )

            # ---------- store ----------
            od = outd.ap().rearrange("n j q c p -> j q n (c p)")
            half = NCH // 2
            for hh in range(2):
                for j in range(4):
                    eng = nc.sync if j % 2 == 0 else nc.scalar
                    eng.dma_start(
                        out=od[j, :, hh * half : (hh + 1) * half, :],
                        in_=out2[32 * j : 32 * j + OUT, hh * half : (hh + 1) * half, :],
                    )

    nc.finalize()
    return nc


def _get_module(cap=32):
    key = ("nc", cap)
    if key not in _CACHE:
        _CACHE[key] = _build_module(cap)
    return _CACHE[key]


def _shard_rois(bidx):
    """Assign ROI indices (grouped by batch) to cores.

    Returns (cap, per-core list of (batch, index-array))."""
    groups = [np.nonzero(bidx == b)[0] for b in range(B)]
    counts = [len(g) for g in groups]
    k = [0, 0]
    for b in range(B):
        if counts[b]:
            k[b] = max(1, int(round(N_CORES * counts[b] / max(sum(counts), 1))))
    while sum(k) > N_CORES:
        b = int(np.argmax([k[0], k[1]]))
        k[b] -= 1
    while sum(k) < N_CORES and max(counts) > 0:
        b = int(np.argmax([counts[b2] / max(k[b2], 0.5) for b2 in range(B)]))
        k[b] += 1
    cap = 8
    for b in range(B):
        if k[b]:
            cap = max(cap, math.ceil(counts[b] / k[b]))
    cap = 4 * math.ceil(cap / 4)
    assert cap * OUT <= 512 // 1 and cap <= 72
    assign = []
    for b in range(B):
        if k[b] == 0:
            continue
        for p in np.array_split(groups[b], k[b]):
            assign.append((b, p))
    while len(assign) < N_CORES:
        assign.append((0, np.zeros((0,), np.int64)))
    return cap, assign[:N_CORES]


def kernel(features, rois):
    features = np.ascontiguousarray(np.asarray(features, dtype=np.float32))
    rois = np.asarray(rois, dtype=np.float32)
    assert features.shape == (B, C, H, W) and rois.shape == (N_ROIS, 5)

    bidx = rois[:, 0].astype(np.int32)
    cap, assign = _shard_rois(bidx)

    nc = _get_module(cap)
    in_maps = []
    for b, idxs in assign:
        rc = np.zeros((cap, 4), np.float32)
        rc[: len(idxs)] = rois[idxs, 1:5]
        in_maps.append(
            {"feat": features[b], "rois": np.ascontiguousarray(rc.reshape(-1))}
        )

    res = run_bass_kernel_spmd(nc, in_maps, list(range(N_CORES)))

    out = np.zeros((N_ROIS, C, OUT, OUT), np.float32)
    for core, (b, idxs) in enumerate(assign):
        if len(idxs) == 0:
            continue
        o = np.asarray(res.results[core]["out"])  # [NCH, 4, q, c, p]
        og = np.transpose(o, (0, 1, 3, 4, 2)).reshape(cap, C, OUT, OUT)
        out[idxs] = og[: len(idxs)]
    return out
